# revision 1
# baseline (speedup 1.0000x reference)
"""Distributed Bass kernel for nn_Attention (B=2, S=2048, HID=2048, H=32, KVH=8, D=64).

Sharding strategy (8 NeuronCores, uniform SPMD graph on every core):
  - Phase 1 (head-parallel): core c owns kv-head c and its 4 GQA query heads.
    x is replicated (transposed to [HID, B*S] bf16 on host); each core computes
    Q^T [256, 4096], K^T [64, 4096] (+RoPE) and V [4096, 64] for its heads.
    No collective needed; causal attention work is identical on every core.
  - Attention in S^T layout [keys, q]: S^T = (K^T-block).T-matmul vs Q^T, exp on
    ACT (scale 1/sqrt(D) folded into Q's RoPE trig tables on host), causal
    handled at 128-key block granularity with two static 0/1 bf16 masks for the
    diagonal blocks, and the softmax denominator comes from a ones-column
    appended to V (row 64 of the AV accumulation).  O^T accumulates in PSUM in
    exactly the [head*D, tokens] layout the output projection needs as lhsT.
  - One AllToAll (2MB/core) re-shards attn_out from head-parallel to
    token-parallel: core c ends with attn^T [2048, 512] for tokens
    [512c, 512(c+1)).
  - Phase 2 (token-parallel): out rows = attn^T.T @ wo (wo replicated, bf16),
    each core writes its own 512 output rows; host-side gather is a pure concat.

Compute dtype: bf16 matmuls (f32 PSUM accumulate), f32 softmax/exp/normalize.
"""

import numpy as np
import ml_dtypes

import concourse.bass as bass
import concourse.mybir as mybir
import concourse.tile as tile
from concourse import bacc
from concourse.bass_utils import run_bass_kernel_spmd

BF16 = ml_dtypes.bfloat16
F32 = np.float32

B, S, HID = 2, 2048, 2048
H, KVH, D = 32, 8, 64
NC = 8                 # cores
T = B * S              # 4096 flat tokens
TL = T // NC           # 512 tokens per core (phase-2 output rows)
LH = H // NC           # 4 local q-heads per core
QC = 256               # query chunk width
KB = 128               # key block
NCH = S // QC          # 8 chunks per batch
TC = 512               # phase-1 token streaming chunk
NTC = T // TC          # 8 token chunks

_CACHE = {}


def _build():
    import os as _os0
    FP8AV = _os0.environ.get("KFP8", "0") == "1"
    fp32 = mybir.dt.float32
    bf16 = mybir.dt.bfloat16
    fp8 = mybir.dt.float8e4

    nc = bacc.Bacc("TRN2", target_bir_lowering=False, debug=False, num_devices=NC)

    xT = nc.dram_tensor("xT", [HID, T], bf16, kind="ExternalInput")
    wq_c = nc.dram_tensor("wq_c", [HID, LH * D], bf16, kind="ExternalInput")
    wkv_c = nc.dram_tensor("wkv_c", [HID, 2 * D], bf16, kind="ExternalInput")
    wo = nc.dram_tensor("wo", [HID, HID], bf16, kind="ExternalInput")
    ctq_d = nc.dram_tensor("ctq", [128, T], fp32, kind="ExternalInput")
    stq_d = nc.dram_tensor("stq", [128, T], fp32, kind="ExternalInput")
    ctk_d = nc.dram_tensor("ctk", [64, T], fp32, kind="ExternalInput")
    stk_d = nc.dram_tensor("stk", [64, T], fp32, kind="ExternalInput")
    m0_d = nc.dram_tensor("m0", [128, 2 * QC], bf16, kind="ExternalInput")
    m1_d = nc.dram_tensor("m1", [128, 2 * QC], bf16, kind="ExternalInput")
    m0a_d = nc.dram_tensor("m0a", [128, QC], fp32, kind="ExternalInput")
    m1a_d = nc.dram_tensor("m1a", [128, QC], fp32, kind="ExternalInput")
    ma_d = [nc.dram_tensor(f"ma{j}", [128, 512], bf16, kind="ExternalInput")
            for j in range(4)]
    out_d = nc.dram_tensor("out", [TL, HID], fp32, kind="ExternalOutput")

    with tile.TileContext(nc) as tc:
        with (
            tc.tile_pool(name="persist", bufs=1) as persist,
            tc.tile_pool(name="wpool", bufs=1) as wpool,
            tc.tile_pool(name="stream", bufs=32) as stream,
            tc.tile_pool(name="trig", bufs=2) as trig,
            tc.tile_pool(name="work", bufs=2) as work,
            tc.tile_pool(name="psum", bufs=1, space="PSUM") as psum,
            tc.tile_pool(name="dram", bufs=1, space="DRAM") as dram,
        ):
            # ---- persistent tiles ----
            qT = [persist.tile([128, T], bf16, tag=f"qT{t}", name=f"qT{t}")
                  for t in range(2)]
            k2 = persist.tile([128, T], bf16, tag="k2", name="k2")
            vatt = [persist.tile([128, D + 1], bf16, tag=f"vatt{i}", name=f"vatt{i}")
                    for i in range(T // KB)]
            vatt8 = [persist.tile([128, 2, 80], fp8, tag=f"vatt8_{i}",
                                  name=f"vatt8_{i}")
                     for i in range(T // (2 * KB))]
            attnT = [persist.tile([128, T], bf16, tag=f"attnT{t}", name=f"attnT{t}")
                     for t in range(2)]
            m0 = persist.tile([128, 2 * QC], bf16, tag="m0", name="m0")
            m1 = persist.tile([128, 2 * QC], bf16, tag="m1", name="m1")
            ident = persist.tile([128, 128], bf16, tag="ident", name="ident")

            nc.sync.dma_start(m0[:], m0_d[:])
            nc.sync.dma_start(m1[:], m1_d[:])
            ma = []
            for j in range(4):
                mt = persist.tile([128, 512], bf16, tag=f"ma{j}", name=f"ma{j}")
                nc.sync.dma_start(mt[:], ma_d[j][:])
                ma.append(mt)
            from concourse.masks import make_identity
            make_identity(nc, ident[:])

            # ---- weights (small per-core shards) ----
            wq_sb = [wpool.tile([128, LH * D], bf16, tag=f"wq{k}", name=f"wq{k}")
                     for k in range(16)]
            wkv_sb = [wpool.tile([128, 2 * D], bf16, tag=f"wkv{k}", name=f"wkv{k}")
                      for k in range(16)]
            for k in range(16):
                nc.sync.dma_start(wq_sb[k][:], wq_c[128 * k:128 * (k + 1), :])
                nc.sync.dma_start(wkv_sb[k][:], wkv_c[128 * k:128 * (k + 1), :])

            # ================= Phase 1: QKV projections + RoPE =================
            def rope(out_ap, ps, ct, st, npart):
                """out = ps*ct + swap32(ps)*st  (st carries the rotate-half sign).

                ps: PSUM [npart, TC] f32; ct/st: SBUF [npart, TC] f32;
                out_ap: bf16 [npart, TC].  npart in {64, 128}.
                """
                t1 = work.tile([128, TC], fp32, tag="rope_t1", name="t1")
                t2 = work.tile([128, TC], fp32, tag="rope_t2", name="t2")
                nc.vector.tensor_mul(t1[:npart, :], ps[:npart, :], ct[:npart, :])
                for base in range(0, npart, 64):
                    a, b = base, base + 32
                    nc.vector.tensor_mul(t2[a:a + 32, :], ps[b:b + 32, :], st[a:a + 32, :])
                    nc.vector.tensor_mul(t2[b:b + 32, :], ps[a:a + 32, :], st[b:b + 32, :])
                nc.vector.tensor_add(out_ap, t1[:npart, :], t2[:npart, :])

            for tc8 in range(NTC):
                tsl = slice(TC * tc8, TC * (tc8 + 1))
                xt = []
                for k in range(16):
                    xk = stream.tile([128, TC], bf16, tag="s", name=f"x{tc8}_{k}")
                    eng = (nc.sync, nc.gpsimd, nc.scalar)[k % 3]
                    eng.dma_start(xk[:], xT[128 * k:128 * (k + 1), tsl])
                    xt.append(xk)
                ctq = trig.tile([128, TC], fp32, tag="ctq", name="ctq")
                stq = trig.tile([128, TC], fp32, tag="stq", name="stq")
                ctk = trig.tile([64, TC], fp32, tag="ctk", name="ctk")
                stk = trig.tile([64, TC], fp32, tag="stk", name="stk")
                nc.scalar.dma_start(ctq[:], ctq_d[:, tsl])
                nc.scalar.dma_start(stq[:], stq_d[:, tsl])
                nc.scalar.dma_start(ctk[:], ctk_d[:, tsl])
                nc.scalar.dma_start(stk[:], stk_d[:, tsl])

                # Q^T: two 128-row tiles (2 heads each)
                for qt in range(2):
                    ps = psum.tile([128, TC], fp32, tag="mm", bufs=2, name="ps_q")
                    for k in range(16):
                        nc.tensor.matmul(ps[:], wq_sb[k][:, 128 * qt:128 * (qt + 1)],
                                         xt[k][:], start=(k == 0), stop=(k == 15))
                    rope(qT[qt][:, tsl], ps, ctq, stq, 128)

                # K^T (rows 0:64) and V^T (rows 64:128) in one packed projection
                ps = psum.tile([128, TC], fp32, tag="mm", bufs=2, name="ps_kv")
                for k in range(16):
                    nc.tensor.matmul(ps[:], wkv_sb[k][:], xt[k][:],
                                     start=(k == 0), stop=(k == 15))
                rope(k2[0:64, tsl], ps, ctk, stk, 64)
                nc.vector.tensor_copy(k2[64:128, tsl], k2[0:64, tsl])

                vt = work.tile([64, TC], bf16, tag="vt", name="vt")
                nc.vector.tensor_copy(vt[:], ps[64:128, :])
                for j in range(TC // KB):
                    kbi = (TC // KB) * tc8 + j
                    pst = psum.tile([128, TC], bf16, tag="mm", bufs=2, name="ps_tr")
                    nc.tensor.transpose(pst[:, 0:64], vt[:, 128 * j:128 * (j + 1)],
                                        ident[0:64, 0:64])
                    nc.vector.tensor_copy(vatt[kbi][:, 0:D], pst[:, 0:64])
                    nc.gpsimd.memset(vatt[kbi][:, D:D + 1], 1.0)

            # wo loads (start draining DMA queues once x loads finish)

            # ================= Attention (head-parallel, causal) ===============
            # Loop (b, cq) outer, head-pair inner: the two pairs' S^T/AV streams
            # interleave so the PE stays dense while ACT runs exp.  Within a
            # pair, the two heads sit on PE row-groups 0/64 (concurrent
            # matmuls), and they share this core's single kv-head, so one
            # AV matmul (N=512) serves both heads.  Softmax denominators (row D
            # of psO, from the ones-column in vatt) are DMA'd raw into the
            # denominator-AllToAll input; normalization happens token-parallel
            # after the AllToAll (one big reciprocal instead of 64 tiny ones).
            import os as _os
            d2a_inA = dram.tile([NC, 2, TL], fp32, tag="d2a_inA", name="d2a_inA")
            d2a_inB = dram.tile([NC, 2, TL], fp32, tag="d2a_inB", name="d2a_inB")
            ATTN_ORDER = _os.environ.get("KORDER", "v1")
            if ATTN_ORDER == "pair":
                # Unit = (batch, chunk, head-pair).  The two heads of a Q^T
                # tile sit on PE row-groups 0/64; emitting their S^T matmuls
                # back-to-back lets the PE run them concurrently (disjoint
                # row groups), halving S^T cycles.  One exp covers
                # 2 kblocks x 2 heads; masks use the doubled [128,512] tiles.
                for b in range(B):
                    for cq in range(NCH):
                        nkb = 2 * (cq + 1)
                        qs = S * b + QC * cq
                        for pair in range(2):
                            qtile = qT[pair]
                            psOa = psum.tile([128, 512], fp32, tag="mm", bufs=2,
                                             name="psOa")[0:D + 1, 0:QC]
                            psOb = psum.tile([128, 512], fp32, tag="mm", bufs=2,
                                             name="psOb")[0:D + 1, 0:QC]
                            for kb in range(0, nkb, 2):
                                psS = psum.tile([128, 4 * QC], fp32, tag="big",
                                                bufs=3, name="psS")
                                ex = work.tile([128, 4 * QC], bf16, tag="ex",
                                               bufs=4, name="ex")
                                for i in range(2):
                                    kpos = S * b + KB * (kb + i)
                                    nc.tensor.matmul(
                                        psS[:, 512 * i:512 * i + QC],
                                        k2[0:64, kpos:kpos + KB],
                                        qtile[0:64, qs:qs + QC],
                                        start=True, stop=True)
                                    nc.tensor.matmul(
                                        psS[:, 512 * i + QC:512 * i + 2 * QC],
                                        k2[64:128, kpos:kpos + KB],
                                        qtile[64:128, qs:qs + QC],
                                        start=True, stop=True)
                                nc.scalar.activation(
                                    ex[:], psS[:], mybir.ActivationFunctionType.Exp)
                                for i in range(2):
                                    kbi = kb + i
                                    ex2 = ex[:, 512 * i:512 * (i + 1)]
                                    if kbi == nkb - 2:
                                        nc.vector.tensor_mul(ex2, ex2, m0[:])
                                    elif kbi == nkb - 1:
                                        nc.vector.tensor_mul(ex2, ex2, m1[:])
                                    vt_ = vatt[(S // KB) * b + kbi][:]
                                    nc.tensor.matmul(
                                        psOa[:], vt_, ex[:, 512 * i:512 * i + QC],
                                        start=(kbi == 0), stop=(kbi == nkb - 1))
                                    nc.tensor.matmul(
                                        psOb[:], vt_,
                                        ex[:, 512 * i + QC:512 * i + 2 * QC],
                                        start=(kbi == 0), stop=(kbi == nkb - 1))
                            for lh, psO in ((2 * pair, psOa), (2 * pair + 1, psOb)):
                                poff = 64 * (lh % 2)
                                nc.vector.tensor_copy(
                                    attnT[pair][poff:poff + 64, qs:qs + QC],
                                    psO[0:D, :])
                                ds = work.tile([1, QC], fp32, tag="ds", bufs=4,
                                               name="ds")
                                nc.vector.tensor_copy(ds[:], psO[D:D + 1, :])
                                jj, off = qs // TL, qs % TL
                                nc.gpsimd.dma_start(
                                    d2a_in[jj, lh, off:off + QC], ds[:])
                units = []
            elif ATTN_ORDER == "v1":
                units = [(lh, b, cq) for lh in range(LH) for b in range(B)
                         for cq in range(4)]
            else:
                units = [(lh, b, cq) for b in range(B) for cq in range(4)
                         for lh in range(LH)]
            for lh, b, cq in units:
                qtile = qT[lh // 2]
                poff = 64 * (lh % 2)
                nkb = 4 * (cq + 1)
                qs = S * b + 512 * cq
                psO = psum.tile([128, 512], fp32, tag="mm", bufs=2,
                                name="psO")[0:D + 1, :]
                for kb in range(0, nkb, 2):
                    psS = psum.tile([128, 1024], fp32, tag="big", bufs=3,
                                    name="psS")
                    ex = work.tile([128, 1024], bf16, tag="ex", bufs=6,
                                   name="ex")
                    for i in range(2):
                        kpos = S * b + KB * (kb + i)
                        nc.tensor.matmul(
                            psS[:, 512 * i:512 * (i + 1)],
                            k2[poff:poff + 64, kpos:kpos + KB],
                            qtile[poff:poff + 64, qs:qs + 512],
                            start=True, stop=True)
                    nc.scalar.activation(ex[:], psS[:],
                                         mybir.ActivationFunctionType.Exp)
                    for i in range(2):
                        kbi = kb + i
                        exs = ex[:, 512 * i:512 * (i + 1)]
                        if kbi >= nkb - 4:
                            nc.vector.tensor_mul(exs, exs, ma[kbi - (nkb - 4)][:])
                        nc.tensor.matmul(
                            psO[:], vatt[(S // KB) * b + kbi][:], exs,
                            start=(kbi == 0), stop=(kbi == nkb - 1))
                # drain: unnormalized O^T slice + raw denominator row
                nc.vector.tensor_copy(
                    attnT[lh // 2][poff:poff + 64, qs:qs + 512], psO[0:D, :])
                ds = work.tile([1, 512], fp32, tag="ds", bufs=4, name="ds")
                nc.vector.tensor_copy(ds[:], psO[D:D + 1, :])
                d2a_t = d2a_inA if lh < 2 else d2a_inB
                nc.gpsimd.dma_start(d2a_t[qs // TL, lh % 2, :], ds[:])

            # ======= AllToAll (split per head-pair for attention overlap) ======
            # With the lh-outer unit order, pair 0 (lh 0,1) finishes its whole
            # attention sweep halfway through; its collectives + normalization
            # run concurrently with pair 1's attention, leaving only the small
            # pair-1 collectives in the serial tail.
            STAGE = int(_os.environ.get("KSTAGE", "4"))
            a2a_in = [dram.tile([NC, 128, TL], bf16, tag=f"a2a_in{t}",
                                name=f"a2a_in{t}") for t in range(2)]
            a2a_out = [dram.tile([NC, 128, TL], bf16, tag=f"a2a_out{t}",
                                 name=f"a2a_out{t}") for t in range(2)]
            d2a_out = [dram.tile([NC, 2, TL], fp32, tag=f"d2a_out{t}",
                                 name=f"d2a_out{t}") for t in range(2)]
            d2a_in = [d2a_inA, d2a_inB]
            rstage = [dram.tile([2 * NC, TL], fp32, tag=f"rstage{t}",
                                name=f"rstage{t}") for t in range(2)]
            ao = {}
            for t in range(2):
                for j in range(NC):
                    nc.sync.dma_start(a2a_in[t][j, :, :],
                                      attnT[t][:, TL * j:TL * (j + 1)])
                if STAGE < 2:
                    continue
                nc.gpsimd.collective_compute(
                    "AllToAll", mybir.AluOpType.bypass,
                    replica_groups=[list(range(NC))],
                    ins=[d2a_in[t].opt()], outs=[d2a_out[t].opt()])
                nc.gpsimd.collective_compute(
                    "AllToAll", mybir.AluOpType.bypass,
                    replica_groups=[list(range(NC))],
                    ins=[a2a_in[t].opt()], outs=[a2a_out[t].opt()])
                den_all = work.tile([2 * NC, TL], fp32, tag="den_all", bufs=2,
                                    name="den_all")
                nc.sync.dma_start(den_all[:],
                                  d2a_out[t].rearrange("a b c -> (a b) c"))
                rall = work.tile([2 * NC, TL], fp32, tag="rall", bufs=2,
                                 name="rall")
                nc.vector.reciprocal(rall[:], den_all[:])
                nc.sync.dma_start(rstage[t][:], rall[:])
                if STAGE < 3:
                    continue
                # normalize this pair's lhsT tiles (ao[kk] for kk%2 == t)
                for r in range(NC):
                    kk = 2 * r + t
                    tl_ = stream.tile([128, TL], bf16, tag="ao", bufs=16,
                                      name=f"ao{kk}")
                    nc.scalar.dma_start(tl_[:], a2a_out[t][r, :, :])
                    rb2 = work.tile([128, TL], fp32, tag="rb2", bufs=2,
                                    name="rb2")
                    nc.gpsimd.dma_start(
                        rb2[0:64, :],
                        rstage[t][2 * r:2 * r + 1, :].broadcast_to([64, TL]))
                    nc.gpsimd.dma_start(
                        rb2[64:128, :],
                        rstage[t][2 * r + 1:2 * r + 2, :].broadcast_to([64, TL]))
                    nc.vector.tensor_mul(tl_[:], tl_[:], rb2[:])
                    ao[kk] = tl_
            if STAGE < 4:
                zb = work.tile([128, 512], fp32, tag="ob", bufs=2, name="zb")
                nc.gpsimd.memset(zb[:], 0.0)
                for tt in range(TL // 128):
                    for nt in range(4):
                        nc.sync.dma_start(out_d[128 * tt:128 * (tt + 1),
                                                512 * nt:512 * (nt + 1)], zb[:])
            for nt in range(4 if STAGE >= 4 else 0):
                wo_sb = []
                for k in range(16):
                    t = stream.tile([128, 512], bf16, tag="s", name=f"wo{k}_{nt}")
                    nc.sync.dma_start(t[:],
                                      wo[128 * k:128 * (k + 1), 512 * nt:512 * (nt + 1)])
                    wo_sb.append(t)
                for tt in range(TL // 128):
                    ps = psum.tile([128, 512], fp32, tag="mm", bufs=2, name="ps_o")
                    for kk in range(16):
                        nc.tensor.matmul(ps[:], ao[kk][:, 128 * tt:128 * (tt + 1)],
                                         wo_sb[kk][:],
                                         start=(kk == 0), stop=(kk == 15))
                    ob = work.tile([128, 512], fp32, tag="ob", bufs=2, name="ob")
                    nc.any.tensor_copy(ob[:], ps[:])
                    nc.gpsimd.dma_start(out_d[128 * tt:128 * (tt + 1),
                                              512 * nt:512 * (nt + 1)], ob[:])

    nc.compile()
    return nc


def _prep_inputs(x, cos, sin, wq, wk, wv, wo):
    x = np.asarray(x, F32)
    cos = np.asarray(cos, F32)
    sin = np.asarray(sin, F32)
    wq = np.asarray(wq, F32)
    wk = np.asarray(wk, F32)
    wv = np.asarray(wv, F32)
    wo = np.asarray(wo, F32)

    xT = np.ascontiguousarray(x.reshape(T, HID).T).astype(BF16)
    wo_b = wo.astype(BF16)

    pos = np.arange(T) % S
    sign = np.concatenate([-np.ones(D // 2, F32), np.ones(D // 2, F32)])
    ctk = np.ascontiguousarray(cos[pos].T)                      # [64, T]
    stk = np.ascontiguousarray((sin[pos] * sign).T)             # [64, T]
    ctq = np.concatenate([ctk, ctk], 0) * F32(1.0 / np.sqrt(D))
    stq = np.concatenate([stk, stk], 0) * F32(1.0 / np.sqrt(D))

    ql = np.arange(QC)
    kl = np.arange(128)
    m0 = (ql[None, :] >= kl[:, None]).astype(BF16)
    m1 = (ql[None, :] >= (kl[:, None] + 128)).astype(BF16)
    m0a = np.where(m0, 0.0, -1e4).astype(F32)   # additive form for fp8 path
    m1a = np.where(m1, 0.0, -1e4).astype(F32)
    qlw = np.arange(512)
    mas = [(qlw[None, :] >= (kl[:, None] + 128 * j)).astype(BF16)
           for j in range(4)]                       # 512-wide diagonal masks
    m0 = np.concatenate([m0, m0], 1)   # [128, 512]: [even head | odd head]
    m1 = np.concatenate([m1, m1], 1)

    in_maps = []
    for c in range(NC):
        wq_c = np.ascontiguousarray(wq[:, c * LH * D:(c + 1) * LH * D]).astype(BF16)
        wkv_c = np.concatenate(
            [wk[:, c * D:(c + 1) * D], wv[:, c * D:(c + 1) * D]], 1).astype(BF16)
        in_maps.append({
            "xT": xT, "wq_c": wq_c, "wkv_c": wkv_c, "wo": wo_b,
            "ctq": ctq, "stq": stq, "ctk": ctk, "stk": stk,
            "m0": m0, "m1": m1, "m0a": m0a, "m1a": m1a,
            "ma0": mas[0], "ma1": mas[1], "ma2": mas[2], "ma3": mas[3],
        })
    return in_maps


def get_nc():
    if "nc" not in _CACHE:
        _CACHE["nc"] = _build()
    return _CACHE["nc"]


def run(in_maps, **kwargs):
    nc = get_nc()
    return run_bass_kernel_spmd(nc, in_maps, core_ids=list(range(NC)), **kwargs)


def kernel(x, cos, sin, wq, wk, wv, wo):
    in_maps = _prep_inputs(x, cos, sin, wq, wk, wv, wo)
    res = run(in_maps)
    out = np.empty((T, HID), F32)
    for c in range(NC):
        out[TL * c:TL * (c + 1)] = res.results[c]["out"]
    return out.reshape(B, S, HID)



# revision 42
# speedup vs baseline: 1.1589x; 1.1589x over previous
"""Distributed Bass kernel for nn_Attention (B=2, S=2048, HID=2048, H=32, KVH=8, D=64).

Sharding (8 cores, uniform SPMD): core c owns kv-head c and its 4 GQA query
heads (2 pairs).  x replicated as xT [HID, T] bf16.

Pipeline (emitted interleaved so all engines overlap):
  per 512-token chunk t (b = t//4, cq = t%4):
    - one 2MB DMA for the x chunk, one DMA per trig table chunk
    - QKV projections (PE) + RoPE (DVE) -> qT/k2/vatt
    - causal attention for (b, cq), both head pairs: per 128-key block one
      psS [128, 2*512] holds both heads' scores (S^T matmuls on disjoint PE
      row groups run concurrently); one EXP (ACT) covers both; AV shares one
      LDWEIGHTS.  Denominator comes from a ones-column in vatt (psO row 64).
  - after every 2 chunks (1024 tokens), an AllToAll redistributes that token
    group's unnormalized attn^T (+2 denominator rows in-band) so every core
    gets its 128-token slice; normalization (reciprocal + broadcast + one
    wide multiply) and the wo projection for those tokens run overlapped
    with the remaining attention.
Output: core c writes out rows [128u + r] = flat token 1024u + 128c + r.
"""

import numpy as np
import ml_dtypes

import concourse.bass as bass
import concourse.mybir as mybir
import concourse.tile as tile
from concourse import bacc
from concourse.bass_utils import run_bass_kernel_spmd

BF16 = ml_dtypes.bfloat16
F32 = np.float32

B, S, HID = 2, 2048, 2048
H, KVH, D = 32, 8, 64
NC = 8
T = B * S              # 4096 flat tokens
LH = H // NC           # 4 local q-heads (2 pairs)
TC = 512               # token chunk
NTC = T // TC          # 8 chunks
KB = 128               # key block
NG = 4                 # a2a token groups (1024 flat tokens each)
GT = T // NG // NC     # 128 tokens per core per group

_CACHE = {}


def _build():
    import os
    DBG = os.environ.get("KDEBUG", "0") == "1"
    fp32 = mybir.dt.float32
    bf16 = mybir.dt.bfloat16

    nc = bacc.Bacc("TRN2", target_bir_lowering=False, debug=False, num_devices=NC)

    xT = nc.dram_tensor("xT", [HID, T], bf16, kind="ExternalInput")
    wq_c = nc.dram_tensor("wq_c", [HID, LH * D], bf16, kind="ExternalInput")
    wkv_c = nc.dram_tensor("wkv_c", [HID, 2 * D], bf16, kind="ExternalInput")
    wo_d = nc.dram_tensor("wo", [HID, HID], bf16, kind="ExternalInput")
    trigq_d = nc.dram_tensor("trigq", [128, 2, T], bf16, kind="ExternalInput")
    trigk_d = nc.dram_tensor("trigk", [64, 2, T], bf16, kind="ExternalInput")
    maD_d = [nc.dram_tensor(f"maD{j}", [128, 2 * TC], bf16, kind="ExternalInput")
             for j in range(4)]
    out_d = nc.dram_tensor("out", [NG * GT, HID], fp32, kind="ExternalOutput")
    if DBG:
        dbg_qT = nc.dram_tensor("dbg_qT", [2, 128, T], fp32, kind="ExternalOutput")
        dbg_k2 = nc.dram_tensor("dbg_k2", [128, T], fp32, kind="ExternalOutput")
        dbg_attnT = nc.dram_tensor("dbg_attnT", [2, 128, T], fp32,
                                   kind="ExternalOutput")
        dbg_den = nc.dram_tensor("dbg_den", [NG, 2, 2, 1024], fp32,
                                 kind="ExternalOutput")
        dbg_ao = nc.dram_tensor("dbg_ao", [NG, 128, 16, GT], fp32,
                                kind="ExternalOutput")
        dbg_rcpb = nc.dram_tensor("dbg_rcpb", [NG, 128, 16, GT], fp32,
                                  kind="ExternalOutput")
        dbg_denall = nc.dram_tensor("dbg_denall", [NG, 32, GT], fp32,
                                    kind="ExternalOutput")
        dbg_rcp = nc.dram_tensor("dbg_rcp", [NG, 32, GT], fp32,
                                 kind="ExternalOutput")
        dbg_a2ain = nc.dram_tensor("dbg_a2ain", [NC, 2, 130, GT], fp32,
                                   kind="ExternalOutput")
        dbg_a2aout = nc.dram_tensor("dbg_a2aout", [NC, 2, 130, GT], fp32,
                                    kind="ExternalOutput")

    with tile.TileContext(nc) as tc:
        with (
            tc.tile_pool(name="persist", bufs=1) as persist,
            tc.tile_pool(name="stream", bufs=2) as stream,
            tc.tile_pool(name="work", bufs=2) as work,
            tc.tile_pool(name="psum", bufs=1, space="PSUM") as psum,
            tc.tile_pool(name="dram", bufs=1, space="DRAM") as dram,
        ):
            # ---- prologue: exp table load warm-up ----
            dummy = work.tile([1, 2], fp32, tag="dummy", name="dummy")
            nc.gpsimd.memset(dummy[:], 0.0)
            nc.scalar.activation(dummy[:], dummy[:],
                                 mybir.ActivationFunctionType.Exp)

            # ---- persistent tiles ----
            qT = [persist.tile([128, T], bf16, tag=f"qT{t}", name=f"qT{t}")
                  for t in range(2)]
            k2 = persist.tile([128, T], bf16, tag="k2", name="k2")
            vatt = [persist.tile([128, D + 1], bf16, tag=f"vatt{i}",
                                 name=f"vatt{i}") for i in range(T // KB)]
            attnT = [persist.tile([128, T], bf16, tag=f"attnT{t}",
                                  name=f"attnT{t}") for t in range(2)]
            den_tiles = {}  # (group u, pair p) -> [1, 2, 1024] bf16 tile
            ident = persist.tile([128, 128], bf16, tag="ident", name="ident")

            # weights: single big DMAs
            wq_sb = persist.tile([128, 16, LH * D], bf16, tag="wq", name="wq")
            wkv_sb = persist.tile([128, 16, 2 * D], bf16, tag="wkv", name="wkv")
            nc.sync.dma_start(
                wq_sb[:], wq_c.rearrange("(k p) m -> p k m", p=128))
            nc.sync.dma_start(
                wkv_sb[:], wkv_c.rearrange("(k p) m -> p k m", p=128))

            maD = []
            for j in range(4):
                mt = persist.tile([128, 2 * TC], bf16, tag=f"maD{j}",
                                  name=f"maD{j}")
                nc.sync.dma_start(mt[:], maD_d[j][:])
                maD.append(mt)
            from concourse.masks import make_identity
            make_identity(nc, ident[:])
            for i in range(T // KB):
                nc.gpsimd.memset(vatt[i][:, D:D + 1], 1.0)

            # wo: 4 column blocks, resident
            wo_sb = []
            for nt in range(4):
                t_ = persist.tile([128, 16, 512], bf16, tag=f"wo{nt}",
                                  name=f"wo{nt}")
                wo_sb.append(t_)

            # a2a staging (one group = 1024 flat tokens; slot j -> core j's
            # 128 tokens; rows 0:128 attnT block, 128:130 the 2 denominators)
            a2a_in = [dram.tile([NC, 2, 130, GT], bf16, tag=f"a2a_in{u}",
                                name=f"a2a_in{u}") for u in range(NG)]
            a2a_out = [dram.tile([NC, 2, 130, GT], bf16, tag=f"a2a_out{u}",
                                 name=f"a2a_out{u}") for u in range(NG)]
            rstage = [dram.tile([2, 16, GT], bf16, tag=f"rstage{u}",
                                name=f"rstage{u}") for u in range(NG)]

            def rope_q(out_ap, ps, tg, tsl):
                ct = tg[:, 0, :]
                st = tg[:, 1, :]
                t1 = work.tile([128, TC], fp32, tag="rope_t1", bufs=1, name="t1")
                t2 = work.tile([128, TC], fp32, tag="rope_t2", bufs=1, name="t2")
                nc.vector.tensor_mul(t1[:], ps[:], ct)
                for base in range(0, 128, 64):
                    a, b = base, base + 32
                    nc.vector.tensor_mul(t2[a:a + 32, :], ps[b:b + 32, :],
                                         st[a:a + 32, :])
                    nc.vector.tensor_mul(t2[b:b + 32, :], ps[a:a + 32, :],
                                         st[b:b + 32, :])
                nc.vector.tensor_add(out_ap, t1[:], t2[:])

            def rope_k(out_ap, ps, tg):
                ct = tg[:, 0, :]
                st = tg[:, 1, :]
                t1 = work.tile([64, TC], fp32, tag="rope_t1", bufs=1, name="kt1")
                t2 = work.tile([64, TC], fp32, tag="rope_t2", bufs=1, name="kt2")
                nc.vector.tensor_mul(t1[:], ps[0:64, :], ct)
                nc.vector.tensor_mul(t2[0:32, :], ps[32:64, :], st[0:32, :])
                nc.vector.tensor_mul(t2[32:64, :], ps[0:32, :], st[32:64, :])
                nc.vector.tensor_add(out_ap, t1[:], t2[:])

            def emit_a2a(u):
                """Send group u (flat tokens [1024u, 1024u+1024)) to all cores."""
                for p in range(2):
                    nc.gpsimd.dma_start(
                        a2a_in[u][:, p, 0:128, :].rearrange("j q t -> q j t"),
                        attnT[p][:, 1024 * u:1024 * (u + 1)]
                        .rearrange("q (j t) -> q j t", j=NC))
                    for h in range(2):
                        nc.gpsimd.dma_start(
                            a2a_in[u][:, p, 128 + h, :],
                            den_tiles[(u, p)][:, 1024 * h:1024 * (h + 1)]
                            .rearrange("o (j t) -> o j t", j=NC))
                nc.gpsimd.collective_compute(
                    "AllToAll", mybir.AluOpType.bypass,
                    replica_groups=[list(range(NC))],
                    ins=[a2a_in[u].opt()], outs=[a2a_out[u].opt()])
                if DBG:
                    for p in range(2):
                        nc.gpsimd.dma_start(
                            dbg_den[u, p, :, :],
                            den_tiles[(u, p)][:, :]
                            .rearrange("o (h t) -> o h t", h=2))
                    if u == 0:
                        nc.gpsimd.dma_start(dbg_a2ain[:], a2a_in[u][:])
                        nc.gpsimd.dma_start(dbg_a2aout[:], a2a_out[u][:])

            def emit_phase2(u):
                """Consume group u: normalize + project 128 token rows."""
                aoall = stream.tile([128, 16, GT], bf16, tag="aoall", bufs=1,
                                    name=f"aoall{u}")
                nc.sync.dma_start(
                    aoall[:],
                    a2a_out[u][:, :, 0:128, :].rearrange("r p q t -> q (r p) t"))
                den_all = work.tile([32, GT], bf16, tag="den_all",
                                    name=f"den_all{u}")
                # den_all rows laid out h-major: row = 16*h + kk
                for h in range(2):
                    nc.sync.dma_start(
                        den_all[16 * h:16 * (h + 1), :],
                        a2a_out[u][:, :, 128 + h, :]
                        .rearrange("r p t -> (r p) t"))
                rcp = work.tile([32, GT], bf16, tag="rcp", name=f"rcp{u}")
                with nc.allow_low_precision(reason="bf16 softmax denominators"):
                    nc.vector.reciprocal(rcp[:], den_all[:])
                nc.sync.dma_start(
                    rstage[u].rearrange("h k t -> (h k) t"), rcp[:])
                rcpb = stream.tile([128, 16, GT], bf16, tag="rcpb", bufs=1,
                                   name=f"rcpb{u}")
                for h in range(2):
                    nc.gpsimd.dma_start(
                        rcpb[64 * h:64 * (h + 1), :, :],
                        rstage[u][h:h + 1, :, :].broadcast_to([64, 16, GT]))
                if DBG:
                    nc.gpsimd.dma_start(dbg_rcpb[u, :, :, :], rcpb[:])
                    nc.gpsimd.dma_start(dbg_denall[u, :, :], den_all[:])
                    nc.gpsimd.dma_start(dbg_rcp[u, :, :], rcp[:])
                nc.vector.tensor_mul(aoall[:], aoall[:], rcpb[:])
                if DBG:
                    nc.gpsimd.dma_start(dbg_ao[u, :, :, :], aoall[:])
                for nt in range(4):
                    ps = psum.tile([128, 512], fp32, tag="mm", bufs=2,
                                   name=f"ps_o{u}_{nt}")
                    for kk in range(16):
                        nc.tensor.matmul(ps[:], aoall[:, kk, :],
                                         wo_sb[nt][:, kk, :],
                                         start=(kk == 0), stop=(kk == 15))
                    ob = work.tile([128, 512], fp32, tag="ob", name=f"ob{u}_{nt}")
                    nc.vector.tensor_copy(ob[:], ps[:])
                    nc.gpsimd.dma_start(
                        out_d[GT * u:GT * (u + 1), 512 * nt:512 * (nt + 1)],
                        ob[:])

            # ================= main interleaved loop =================
            for t8 in range(NTC):
                b, cq = t8 // 4, t8 % 4
                tsl = slice(TC * t8, TC * (t8 + 1))

                xq = stream.tile([128, 16, TC], bf16, tag="xq", name=f"xq{t8}")
                nc.sync.dma_start(
                    xq[:], xT[:, tsl].rearrange("(k p) t -> p k t", p=128))
                tgq = stream.tile([128, 2, TC], bf16, tag="tgq", bufs=1,
                                  name=f"tgq{t8}")
                tgk = stream.tile([64, 2, TC], bf16, tag="tgk", bufs=1,
                                  name=f"tgk{t8}")
                nc.sync.dma_start(tgq[:], trigq_d[:, :, tsl])
                nc.sync.dma_start(tgk[:], trigk_d[:, :, tsl])
                if t8 < 2:
                    for nt in (2 * t8, 2 * t8 + 1):
                        nc.sync.dma_start(
                            wo_sb[nt][:],
                            wo_d[:, 512 * nt:512 * (nt + 1)]
                            .rearrange("(k p) m -> p k m", p=128))

                # ---- QKV projections + RoPE ----
                for qt in range(2):
                    ps = psum.tile([128, TC], fp32, tag="mm", bufs=2,
                                   name=f"ps_q{t8}_{qt}")
                    for k in range(16):
                        nc.tensor.matmul(ps[:],
                                         wq_sb[:, k, 128 * qt:128 * (qt + 1)],
                                         xq[:, k, :],
                                         start=(k == 0), stop=(k == 15))
                    rope_q(qT[qt][:, tsl], ps, tgq, tsl)

                ps = psum.tile([128, TC], fp32, tag="mm", bufs=2,
                               name=f"ps_kv{t8}")
                for k in range(16):
                    nc.tensor.matmul(ps[:], wkv_sb[:, k, :], xq[:, k, :],
                                     start=(k == 0), stop=(k == 15))
                rope_k(k2[0:64, tsl], ps, tgk)
                nc.vector.tensor_copy(k2[64:128, tsl], k2[0:64, tsl])
                vt = work.tile([64, TC], bf16, tag="vt", name=f"vt{t8}")
                nc.vector.tensor_copy(vt[:], ps[64:128, :])
                for j in range(TC // KB):
                    kbi = (TC // KB) * t8 + j
                    pst = psum.tile([128, TC], bf16, tag="mm", bufs=2,
                                    name=f"ps_tr{t8}_{j}")
                    nc.tensor.transpose(pst[:, 0:64],
                                        vt[:, 128 * j:128 * (j + 1)],
                                        ident[0:64, 0:64])
                    nc.vector.tensor_copy(vatt[kbi][:, 0:D], pst[:, 0:64])

                # ---- attention for (b, cq), both pairs ----
                nkb = 4 * (cq + 1)
                qs = S * b + TC * cq
                gu, goff = qs // 1024, qs % 1024
                for p in range(2):
                    if (gu, p) not in den_tiles:
                        den_tiles[(gu, p)] = work.tile(
                            [1, 2048], bf16, tag=f"deng{p}", bufs=2,
                            name=f"deng{gu}_{p}")
                    qtile = qT[p]
                    psO = psum.tile([D + 1, 1024], fp32, tag="psO", bufs=1,
                                    name=f"psO{t8}_{p}")
                    for kb in range(nkb):
                        kpos = S * b + KB * kb
                        psS = psum.tile([128, 1024], fp32, tag="psS", bufs=2,
                                        name=f"psS{t8}_{p}_{kb}")
                        nc.tensor.matmul(psS[:, 0:512],
                                         k2[0:64, kpos:kpos + KB],
                                         qtile[0:64, qs:qs + TC],
                                         start=True, stop=True)
                        nc.tensor.matmul(psS[:, 512:1024],
                                         k2[64:128, kpos:kpos + KB],
                                         qtile[64:128, qs:qs + TC],
                                         start=True, stop=True)
                        ex = work.tile([128, 1024], bf16, tag="ex", bufs=3,
                                       name=f"ex{t8}_{p}_{kb}")
                        nc.scalar.activation(ex[:], psS[:],
                                             mybir.ActivationFunctionType.Exp)
                        if kb >= nkb - 4:
                            nc.vector.tensor_mul(ex[:], ex[:],
                                                 maD[kb - (nkb - 4)][:])
                        vt_ = vatt[(S // KB) * b + kb][:]
                        nc.tensor.matmul(psO[:, 0:512], vt_, ex[:, 0:512],
                                         start=(kb == 0), stop=(kb == nkb - 1))
                        nc.tensor.matmul(psO[:, 512:1024], vt_, ex[:, 512:1024],
                                         start=(kb == 0), stop=(kb == nkb - 1))
                    # drain
                    nc.vector.tensor_copy(attnT[p][0:64, qs:qs + TC],
                                          psO[0:64, 0:512])
                    nc.vector.tensor_copy(attnT[p][64:128, qs:qs + TC],
                                          psO[0:64, 512:1024])
                    for h in range(2):
                        nc.vector.tensor_copy(
                            den_tiles[(gu, p)][:, 1024 * h + goff:
                                               1024 * h + goff + TC],
                            psO[64:65, 512 * h:512 * (h + 1)])

                if t8 % 2 == 1:
                    emit_a2a(t8 // 2)
                if t8 >= 2 and t8 % 2 == 0:
                    emit_phase2(t8 // 2 - 1)
            emit_phase2(2)
            emit_phase2(3)
            if DBG:
                for p in range(2):
                    nc.gpsimd.dma_start(dbg_qT[p, :, :], qT[p][:])
                    nc.gpsimd.dma_start(dbg_attnT[p, :, :], attnT[p][:])
                nc.gpsimd.dma_start(dbg_k2[:, :], k2[:])

    nc.compile()
    return nc


def _prep_inputs(x, cos, sin, wq, wk, wv, wo):
    x = np.asarray(x, F32)
    cos = np.asarray(cos, F32)
    sin = np.asarray(sin, F32)
    wq = np.asarray(wq, F32)
    wk = np.asarray(wk, F32)
    wv = np.asarray(wv, F32)
    wo = np.asarray(wo, F32)

    xT = np.ascontiguousarray(x.reshape(T, HID).T).astype(BF16)
    wo_b = wo.astype(BF16)

    pos = np.arange(T) % S
    sign = np.concatenate([-np.ones(D // 2, F32), np.ones(D // 2, F32)])
    ctk = np.ascontiguousarray(cos[pos].T)                      # [64, T]
    stk = np.ascontiguousarray((sin[pos] * sign).T)             # [64, T]
    sc = F32(1.0 / np.sqrt(D))
    ctq = np.concatenate([ctk, ctk], 0) * sc                    # [128, T]
    stq = np.concatenate([stk, stk], 0) * sc
    trigq = np.stack([ctq, stq], axis=1).astype(BF16)           # [128, 2, T]
    trigk = np.stack([ctk, stk], axis=1).astype(BF16)           # [64, 2, T]

    ql = np.arange(TC)
    kl = np.arange(128)
    maD = []
    for j in range(4):
        m = (ql[None, :] >= (kl[:, None] + 128 * j)).astype(BF16)
        maD.append(np.concatenate([m, m], axis=1))              # [128, 1024]

    in_maps = []
    for c in range(NC):
        wq_cc = np.ascontiguousarray(
            wq[:, c * LH * D:(c + 1) * LH * D]).astype(BF16)
        wkv_cc = np.concatenate(
            [wk[:, c * D:(c + 1) * D], wv[:, c * D:(c + 1) * D]], 1).astype(BF16)
        in_maps.append({
            "xT": xT, "wq_c": wq_cc, "wkv_c": wkv_cc, "wo": wo_b,
            "trigq": trigq, "trigk": trigk,
            "maD0": maD[0], "maD1": maD[1], "maD2": maD[2], "maD3": maD[3],
        })
    return in_maps


def get_nc():
    if "nc" not in _CACHE:
        _CACHE["nc"] = _build()
    return _CACHE["nc"]


def run(in_maps, **kwargs):
    nc = get_nc()
    return run_bass_kernel_spmd(nc, in_maps, core_ids=list(range(NC)), **kwargs)


def kernel(x, cos, sin, wq, wk, wv, wo):
    in_maps = _prep_inputs(x, cos, sin, wq, wk, wv, wo)
    res = run(in_maps)
    out = np.empty((T, HID), F32)
    for c in range(NC):
        r = res.results[c]["out"]
        for u in range(NG):
            out[1024 * u + GT * c:1024 * u + GT * (c + 1)] = \
                r[GT * u:GT * (u + 1)]
    return out.reshape(B, S, HID)


# revision 46
# speedup vs baseline: 1.3025x; 1.1239x over previous
"""Distributed Bass kernel for nn_Attention (B=2, S=2048, HID=2048, H=32, KVH=8, D=64).

Sharding (8 cores, uniform SPMD): core c owns kv-head c and its 4 GQA query
heads (2 pairs).  x replicated as xT [HID, T] bf16.

Pipeline (emitted interleaved so all engines overlap):
  per 512-token chunk t (b = t//4, cq = t%4):
    - one 2MB DMA for the x chunk, one DMA per trig table chunk
    - QKV projections (PE) + RoPE (DVE) -> qT/k2/vatt
    - causal attention for (b, cq), both head pairs: per 128-key block one
      psS [128, 2*512] holds both heads' scores (S^T matmuls on disjoint PE
      row groups run concurrently); one EXP (ACT) covers both; AV shares one
      LDWEIGHTS.  Denominator comes from a ones-column in vatt (psO row 64).
  - after every 2 chunks (1024 tokens), an AllToAll redistributes that token
    group's unnormalized attn^T (+2 denominator rows in-band) so every core
    gets its 128-token slice; normalization (reciprocal + broadcast + one
    wide multiply) and the wo projection for those tokens run overlapped
    with the remaining attention.
Output: core c writes out rows [128u + r] = flat token 1024u + 128c + r.
"""

import numpy as np
import ml_dtypes

import concourse.bass as bass
import concourse.mybir as mybir
import concourse.tile as tile
from concourse import bacc
from concourse.bass_utils import run_bass_kernel_spmd

BF16 = ml_dtypes.bfloat16
F32 = np.float32

B, S, HID = 2, 2048, 2048
H, KVH, D = 32, 8, 64
NC = 8
T = B * S              # 4096 flat tokens
LH = H // NC           # 4 local q-heads (2 pairs)
TC = 512               # token chunk
NTC = T // TC          # 8 chunks
KB = 128               # key block
NG = 4                 # a2a token groups (1024 flat tokens each)
GT = T // NG // NC     # 128 tokens per core per group

_CACHE = {}


def _build():
    import os
    DBG = os.environ.get("KDEBUG", "0") == "1"
    fp32 = mybir.dt.float32
    bf16 = mybir.dt.bfloat16

    nc = bacc.Bacc("TRN2", target_bir_lowering=False, debug=False, num_devices=NC)

    xT = nc.dram_tensor("xT", [HID, T], bf16, kind="ExternalInput")
    wq_c = nc.dram_tensor("wq_c", [HID, LH * D], bf16, kind="ExternalInput")
    wkv_c = nc.dram_tensor("wkv_c", [HID, 2 * D], bf16, kind="ExternalInput")
    wo_d = nc.dram_tensor("wo", [HID, HID], bf16, kind="ExternalInput")
    trigq_d = nc.dram_tensor("trigq", [128, 2, T], bf16, kind="ExternalInput")
    trigk_d = nc.dram_tensor("trigk", [64, 2, T], bf16, kind="ExternalInput")
    maD_d = [nc.dram_tensor(f"maD{j}", [128, 2 * TC], bf16, kind="ExternalInput")
             for j in range(4)]
    out_d = nc.dram_tensor("out", [NG * GT, HID], fp32, kind="ExternalOutput")
    if DBG:
        dbg_qT = nc.dram_tensor("dbg_qT", [2, 128, T], fp32, kind="ExternalOutput")
        dbg_k2 = nc.dram_tensor("dbg_k2", [128, T], fp32, kind="ExternalOutput")
        dbg_attnT = nc.dram_tensor("dbg_attnT", [2, 128, T], fp32,
                                   kind="ExternalOutput")
        dbg_den = nc.dram_tensor("dbg_den", [NG, 2, 2, 1024], fp32,
                                 kind="ExternalOutput")
        dbg_ao = nc.dram_tensor("dbg_ao", [NG, 128, 16, GT], fp32,
                                kind="ExternalOutput")
        dbg_rcpb = nc.dram_tensor("dbg_rcpb", [NG, 128, 16, GT], fp32,
                                  kind="ExternalOutput")
        dbg_denall = nc.dram_tensor("dbg_denall", [NG, 32, GT], fp32,
                                    kind="ExternalOutput")
        dbg_rcp = nc.dram_tensor("dbg_rcp", [NG, 32, GT], fp32,
                                 kind="ExternalOutput")
        dbg_a2ain = nc.dram_tensor("dbg_a2ain", [NC, 2, 130, GT], fp32,
                                   kind="ExternalOutput")
        dbg_a2aout = nc.dram_tensor("dbg_a2aout", [NC, 2, 130, GT], fp32,
                                    kind="ExternalOutput")

    with tile.TileContext(nc) as tc:
        with (
            tc.tile_pool(name="persist", bufs=1) as persist,
            tc.tile_pool(name="stream", bufs=2) as stream,
            tc.tile_pool(name="work", bufs=2) as work,
            tc.tile_pool(name="psum", bufs=1, space="PSUM") as psum,
            tc.tile_pool(name="dram", bufs=1, space="DRAM") as dram,
        ):
            # ---- prologue: exp table load warm-up ----
            dummy = work.tile([1, 2], fp32, tag="dummy", name="dummy")
            nc.gpsimd.memset(dummy[:], 0.0)
            nc.scalar.activation(dummy[:], dummy[:],
                                 mybir.ActivationFunctionType.Exp)

            # ---- persistent tiles ----
            qT = [persist.tile([128, T], bf16, tag=f"qT{t}", name=f"qT{t}")
                  for t in range(2)]
            k2 = persist.tile([128, T], bf16, tag="k2", name="k2")
            vatt = [persist.tile([128, D + 1], bf16, tag=f"vatt{i}",
                                 name=f"vatt{i}") for i in range(T // KB)]
            attnT = [persist.tile([128, T], bf16, tag=f"attnT{t}",
                                  name=f"attnT{t}") for t in range(2)]
            den_tiles = {}  # (group u, pair p) -> [1, 2, 1024] bf16 tile
            ident = persist.tile([128, 128], bf16, tag="ident", name="ident")

            # weights: single big DMAs
            wq_sb = persist.tile([128, 16, LH * D], bf16, tag="wq", name="wq")
            wkv_sb = persist.tile([128, 16, 2 * D], bf16, tag="wkv", name="wkv")
            nc.sync.dma_start(
                wq_sb[:], wq_c.rearrange("(k p) m -> p k m", p=128))
            nc.sync.dma_start(
                wkv_sb[:], wkv_c.rearrange("(k p) m -> p k m", p=128))

            maD = []
            for j in range(4):
                mt = persist.tile([128, 2 * TC], bf16, tag=f"maD{j}",
                                  name=f"maD{j}")
                nc.sync.dma_start(mt[:], maD_d[j][:])
                maD.append(mt)
            from concourse.masks import make_identity
            make_identity(nc, ident[:])
            for i in range(T // KB):
                nc.gpsimd.memset(vatt[i][:, D:D + 1], 1.0)

            # wo: 4 column blocks, resident
            wo_sb = []
            for nt in range(4):
                t_ = persist.tile([128, 16, 512], bf16, tag=f"wo{nt}",
                                  name=f"wo{nt}")
                wo_sb.append(t_)

            # a2a staging (one group = 1024 flat tokens; slot j -> core j's
            # 128 tokens; rows 0:128 attnT block, 128:130 the 2 denominators)
            a2a_in = [dram.tile([NC, 2, 130, GT], bf16, tag=f"a2a_in{u}",
                                name=f"a2a_in{u}") for u in range(NG)]
            a2a_out = [dram.tile([NC, 2, 130, GT], bf16, tag=f"a2a_out{u}",
                                 name=f"a2a_out{u}") for u in range(NG)]
            rstage = [dram.tile([2, 16, GT], bf16, tag=f"rstage{u}",
                                name=f"rstage{u}") for u in range(NG)]

            def rope_q(out_ap, ps, tg, tsl):
                ct = tg[:, 0, :]
                st = tg[:, 1, :]
                t1 = work.tile([128, TC], fp32, tag="rope_t1", bufs=1, name="t1")
                t2 = work.tile([128, TC], fp32, tag="rope_t2", bufs=1, name="t2")
                nc.vector.tensor_mul(t1[:], ps[:], ct)
                for base in range(0, 128, 64):
                    a, b = base, base + 32
                    nc.vector.tensor_mul(t2[a:a + 32, :], ps[b:b + 32, :],
                                         st[a:a + 32, :])
                    nc.vector.tensor_mul(t2[b:b + 32, :], ps[a:a + 32, :],
                                         st[b:b + 32, :])
                nc.vector.tensor_add(out_ap, t1[:], t2[:])

            def rope_k(out_ap, ps, tg):
                ct = tg[:, 0, :]
                st = tg[:, 1, :]
                t1 = work.tile([64, TC], fp32, tag="rope_t1", bufs=1, name="kt1")
                t2 = work.tile([64, TC], fp32, tag="rope_t2", bufs=1, name="kt2")
                nc.vector.tensor_mul(t1[:], ps[0:64, :], ct)
                nc.vector.tensor_mul(t2[0:32, :], ps[32:64, :], st[0:32, :])
                nc.vector.tensor_mul(t2[32:64, :], ps[0:32, :], st[32:64, :])
                nc.vector.tensor_add(out_ap, t1[:], t2[:])

            def emit_a2a(u):
                """Send group u (flat tokens [1024u, 1024u+1024)) to all cores."""
                for p in range(2):
                    nc.gpsimd.dma_start(
                        a2a_in[u][:, p, 0:128, :].rearrange("j q t -> q j t"),
                        attnT[p][:, 1024 * u:1024 * (u + 1)]
                        .rearrange("q (j t) -> q j t", j=NC))
                    for h in range(2):
                        nc.gpsimd.dma_start(
                            a2a_in[u][:, p, 128 + h, :],
                            den_tiles[(u, p)][:, 1024 * h:1024 * (h + 1)]
                            .rearrange("o (j t) -> o j t", j=NC))
                nc.gpsimd.collective_compute(
                    "AllToAll", mybir.AluOpType.bypass,
                    replica_groups=[list(range(NC))],
                    ins=[a2a_in[u].opt()], outs=[a2a_out[u].opt()])
                if DBG:
                    for p in range(2):
                        nc.gpsimd.dma_start(
                            dbg_den[u, p, :, :],
                            den_tiles[(u, p)][:, :]
                            .rearrange("o (h t) -> o h t", h=2))
                    if u == 0:
                        nc.gpsimd.dma_start(dbg_a2ain[:], a2a_in[u][:])
                        nc.gpsimd.dma_start(dbg_a2aout[:], a2a_out[u][:])

            p2_state = {}

            def emit_phase2(u, half):
                """Consume group u: normalize + project 128 token rows.
                half 0 = normalization + nt 0,1; half 1 = nt 2,3."""
                if half == 0:
                    p2_state[u] = _emit_phase2_setup(u)
                aoall = p2_state[u]
                for nt in (0, 1) if half == 0 else (2, 3):
                    ps = psum.tile([128, 512], fp32, tag="mm", bufs=2,
                                   name=f"ps_o{u}_{nt}")
                    for kk in range(16):
                        nc.tensor.matmul(ps[:], aoall[:, kk, :],
                                         wo_sb[nt][:, kk, :],
                                         start=(kk == 0), stop=(kk == 15))
                    ob = work.tile([128, 512], fp32, tag="ob", name=f"ob{u}_{nt}")
                    nc.vector.tensor_copy(ob[:], ps[:])
                    nc.gpsimd.dma_start(
                        out_d[GT * u:GT * (u + 1), 512 * nt:512 * (nt + 1)],
                        ob[:])

            def _emit_phase2_setup(u):
                aoall = stream.tile([128, 16, GT], bf16, tag="aoall", bufs=1,
                                    name=f"aoall{u}")
                nc.sync.dma_start(
                    aoall[:],
                    a2a_out[u][:, :, 0:128, :].rearrange("r p q t -> q (r p) t"))
                den_all = work.tile([32, GT], bf16, tag="den_all",
                                    name=f"den_all{u}")
                # den_all rows laid out h-major: row = 16*h + kk
                for h in range(2):
                    nc.sync.dma_start(
                        den_all[16 * h:16 * (h + 1), :],
                        a2a_out[u][:, :, 128 + h, :]
                        .rearrange("r p t -> (r p) t"))
                rcp = work.tile([32, GT], bf16, tag="rcp", name=f"rcp{u}")
                with nc.allow_low_precision(reason="bf16 softmax denominators"):
                    nc.vector.reciprocal(rcp[:], den_all[:])
                nc.sync.dma_start(
                    rstage[u].rearrange("h k t -> (h k) t"), rcp[:])
                rcpb = stream.tile([128, 16, GT], bf16, tag="rcpb", bufs=1,
                                   name=f"rcpb{u}")
                for h in range(2):
                    nc.gpsimd.dma_start(
                        rcpb[64 * h:64 * (h + 1), :, :],
                        rstage[u][h:h + 1, :, :].broadcast_to([64, 16, GT]))
                if DBG:
                    nc.gpsimd.dma_start(dbg_rcpb[u, :, :, :], rcpb[:])
                    nc.gpsimd.dma_start(dbg_denall[u, :, :], den_all[:])
                    nc.gpsimd.dma_start(dbg_rcp[u, :, :], rcp[:])
                nc.vector.tensor_mul(aoall[:], aoall[:], rcpb[:])
                if DBG:
                    nc.gpsimd.dma_start(dbg_ao[u, :, :, :], aoall[:])
                return aoall

            # ================= main interleaved loop =================
            for t8 in range(NTC):
                b, cq = t8 // 4, t8 % 4
                tsl = slice(TC * t8, TC * (t8 + 1))

                xq = stream.tile([128, 16, TC], bf16, tag="xq", name=f"xq{t8}")
                nc.sync.dma_start(
                    xq[:], xT[:, tsl].rearrange("(k p) t -> p k t", p=128))
                tgq = stream.tile([128, 2, TC], bf16, tag="tgq", bufs=1,
                                  name=f"tgq{t8}")
                tgk = stream.tile([64, 2, TC], bf16, tag="tgk", bufs=1,
                                  name=f"tgk{t8}")
                nc.sync.dma_start(tgq[:], trigq_d[:, :, tsl])
                nc.sync.dma_start(tgk[:], trigk_d[:, :, tsl])
                if t8 < 4:
                    nc.sync.dma_start(
                        wo_sb[t8][:],
                        wo_d[:, 512 * t8:512 * (t8 + 1)]
                        .rearrange("(k p) m -> p k m", p=128))

                # ---- QKV projections + RoPE ----
                for qt in range(2):
                    ps = psum.tile([128, TC], fp32, tag="mm", bufs=2,
                                   name=f"ps_q{t8}_{qt}")
                    for k in range(16):
                        nc.tensor.matmul(ps[:],
                                         wq_sb[:, k, 128 * qt:128 * (qt + 1)],
                                         xq[:, k, :],
                                         start=(k == 0), stop=(k == 15))
                    rope_q(qT[qt][:, tsl], ps, tgq, tsl)

                ps = psum.tile([128, TC], fp32, tag="mm", bufs=2,
                               name=f"ps_kv{t8}")
                for k in range(16):
                    nc.tensor.matmul(ps[:], wkv_sb[:, k, :], xq[:, k, :],
                                     start=(k == 0), stop=(k == 15))
                rope_k(k2[0:64, tsl], ps, tgk)
                nc.vector.tensor_copy(k2[64:128, tsl], k2[0:64, tsl])
                vt = work.tile([64, TC], bf16, tag="vt", name=f"vt{t8}")
                nc.vector.tensor_copy(vt[:], ps[64:128, :])
                for j in range(TC // KB):
                    kbi = (TC // KB) * t8 + j
                    pst = psum.tile([128, TC], bf16, tag="mm", bufs=2,
                                    name=f"ps_tr{t8}_{j}")
                    nc.tensor.transpose(pst[:, 0:64],
                                        vt[:, 128 * j:128 * (j + 1)],
                                        ident[0:64, 0:64])
                    nc.vector.tensor_copy(vatt[kbi][:, 0:D], pst[:, 0:64])

                # ---- attention for (b, cq), both pairs ----
                nkb = 4 * (cq + 1)
                qs = S * b + TC * cq
                gu, goff = qs // 1024, qs % 1024
                for p in range(2):
                    if (gu, p) not in den_tiles:
                        den_tiles[(gu, p)] = work.tile(
                            [1, 2048], bf16, tag=f"deng{p}", bufs=2,
                            name=f"deng{gu}_{p}")
                    qtile = qT[p]
                    psO = psum.tile([D + 1, 1024], fp32, tag="psO", bufs=1,
                                    name=f"psO{t8}_{p}")
                    for kb in range(nkb):
                        kpos = S * b + KB * kb
                        psS = psum.tile([128, 1024], fp32, tag="psS", bufs=2,
                                        name=f"psS{t8}_{p}_{kb}")
                        nc.tensor.matmul(psS[:, 0:512],
                                         k2[0:64, kpos:kpos + KB],
                                         qtile[0:64, qs:qs + TC],
                                         start=True, stop=True)
                        nc.tensor.matmul(psS[:, 512:1024],
                                         k2[64:128, kpos:kpos + KB],
                                         qtile[64:128, qs:qs + TC],
                                         start=True, stop=True)
                        ex = work.tile([128, 1024], bf16, tag="ex", bufs=3,
                                       name=f"ex{t8}_{p}_{kb}")
                        nc.scalar.activation(ex[:], psS[:],
                                             mybir.ActivationFunctionType.Exp)
                        if kb >= nkb - 4:
                            nc.vector.tensor_mul(ex[:], ex[:],
                                                 maD[kb - (nkb - 4)][:])
                        vt_ = vatt[(S // KB) * b + kb][:]
                        nc.tensor.matmul(psO[:, 0:512], vt_, ex[:, 0:512],
                                         start=(kb == 0), stop=(kb == nkb - 1))
                        nc.tensor.matmul(psO[:, 512:1024], vt_, ex[:, 512:1024],
                                         start=(kb == 0), stop=(kb == nkb - 1))
                    # drain
                    nc.vector.tensor_copy(attnT[p][0:64, qs:qs + TC],
                                          psO[0:64, 0:512])
                    nc.vector.tensor_copy(attnT[p][64:128, qs:qs + TC],
                                          psO[0:64, 512:1024])
                    for h in range(2):
                        nc.vector.tensor_copy(
                            den_tiles[(gu, p)][:, 1024 * h + goff:
                                               1024 * h + goff + TC],
                            psO[64:65, 512 * h:512 * (h + 1)])
                    if t8 >= 3 and t8 % 2 == 1:
                        emit_phase2((t8 - 3) // 2, p)

                if t8 % 2 == 1:
                    emit_a2a(t8 // 2)
            emit_phase2(3, 0)
            emit_phase2(3, 1)
            if DBG:
                for p in range(2):
                    nc.gpsimd.dma_start(dbg_qT[p, :, :], qT[p][:])
                    nc.gpsimd.dma_start(dbg_attnT[p, :, :], attnT[p][:])
                nc.gpsimd.dma_start(dbg_k2[:, :], k2[:])

    nc.compile()
    return nc


def _prep_inputs(x, cos, sin, wq, wk, wv, wo):
    x = np.asarray(x, F32)
    cos = np.asarray(cos, F32)
    sin = np.asarray(sin, F32)
    wq = np.asarray(wq, F32)
    wk = np.asarray(wk, F32)
    wv = np.asarray(wv, F32)
    wo = np.asarray(wo, F32)

    xT = np.ascontiguousarray(x.reshape(T, HID).T).astype(BF16)
    wo_b = wo.astype(BF16)

    pos = np.arange(T) % S
    sign = np.concatenate([-np.ones(D // 2, F32), np.ones(D // 2, F32)])
    ctk = np.ascontiguousarray(cos[pos].T)                      # [64, T]
    stk = np.ascontiguousarray((sin[pos] * sign).T)             # [64, T]
    sc = F32(1.0 / np.sqrt(D))
    ctq = np.concatenate([ctk, ctk], 0) * sc                    # [128, T]
    stq = np.concatenate([stk, stk], 0) * sc
    trigq = np.stack([ctq, stq], axis=1).astype(BF16)           # [128, 2, T]
    trigk = np.stack([ctk, stk], axis=1).astype(BF16)           # [64, 2, T]

    ql = np.arange(TC)
    kl = np.arange(128)
    maD = []
    for j in range(4):
        m = (ql[None, :] >= (kl[:, None] + 128 * j)).astype(BF16)
        maD.append(np.concatenate([m, m], axis=1))              # [128, 1024]

    in_maps = []
    for c in range(NC):
        wq_cc = np.ascontiguousarray(
            wq[:, c * LH * D:(c + 1) * LH * D]).astype(BF16)
        wkv_cc = np.concatenate(
            [wk[:, c * D:(c + 1) * D], wv[:, c * D:(c + 1) * D]], 1).astype(BF16)
        in_maps.append({
            "xT": xT, "wq_c": wq_cc, "wkv_c": wkv_cc, "wo": wo_b,
            "trigq": trigq, "trigk": trigk,
            "maD0": maD[0], "maD1": maD[1], "maD2": maD[2], "maD3": maD[3],
        })
    return in_maps


def get_nc():
    if "nc" not in _CACHE:
        _CACHE["nc"] = _build()
    return _CACHE["nc"]


def run(in_maps, **kwargs):
    nc = get_nc()
    return run_bass_kernel_spmd(nc, in_maps, core_ids=list(range(NC)), **kwargs)


def kernel(x, cos, sin, wq, wk, wv, wo):
    in_maps = _prep_inputs(x, cos, sin, wq, wk, wv, wo)
    res = run(in_maps)
    out = np.empty((T, HID), F32)
    for c in range(NC):
        r = res.results[c]["out"]
        for u in range(NG):
            out[1024 * u + GT * c:1024 * u + GT * (c + 1)] = \
                r[GT * u:GT * (u + 1)]
    return out.reshape(B, S, HID)
